# revision 2
# baseline (speedup 1.0000x reference)
"""GCN link predictor on 8 trn2 NeuronCores (Bass/Tile).

Sharding: nodes row-sharded across 8 cores (dst-partitioned edges so the
segment-sum stays local); per-layer full feature tables shared via AllGather;
decode edges sharded data-parallel.

Math: gcn_conv(x) = D^-1/2 (A+I) D^-1/2 (x W) + b.  We fold D^-1/2 into the
tables: h' = (x W) * dinv[row], out[d] = dinv[d] * sum_{s in N(d)} h'[s] + b.
The per-edge norm disappears; aggregation is a 0/1 select-matrix matmul on the
TensorEngine over edges gathered by dma_gather (512B rows at HBM line rate).
The +b is a rank-1 matmul (sqrt(deg)[d] x b) folded into the same PSUM
accumulation, so the epilogue is one ScalarE activation (scale=dinv, Relu/Id).
"""
import math
import os
import numpy as np
from contextlib import ExitStack

import concourse.bass as bass
import concourse.bacc as bacc
import concourse.tile as tile
import concourse.mybir as mybir
from concourse.bass_utils import run_bass_kernel_spmd

F32 = mybir.dt.float32
I16 = mybir.dt.int16


def _rows_ap(handle, row0: int, nblk: int, width: int):
    """DRAM [rows, width] view as [128, nblk, width]: p,b,f -> row row0+b*128+p."""
    a = handle[:]
    return bass.AP(tensor=a.tensor, offset=row0 * width,
                   ap=[[width, 128], [128 * width, nblk], [1, width]])
NCORES = 8
P = 128
CHUNK = 32768          # int16-addressable rows per gather table chunk
B_TILES = 6            # dst tiles per aggregation batch (PSUM banks: 6+1+1)
RING_G = int(os.environ.get("KRING", "4"))
                       # msg ring slot size in 128-edge groups (HW: >1024 idxs
                       # per dma_gather faults; 512 adds margin vs flakiness)
DEC_RING = int(os.environ.get("KDRING", "4"))  # decode ring slot size in groups


# ---------------------------------------------------------------- host prep

def _pack_idx(stream_i16: np.ndarray) -> np.ndarray:
    """int16 stream -> [128, L/16] tile (pos i -> [i%16, i//16], x8 replicated)."""
    L = stream_i16.shape[0]
    assert L % 16 == 0
    a16 = stream_i16.reshape(L // 16, 16).T.copy()
    return np.tile(a16, (8, 1))


def _pack_f32(stream_f32: np.ndarray) -> np.ndarray:
    """f32 stream -> [128, L/128] tile (pos i -> [i%128, i//128])."""
    L = stream_f32.shape[0]
    assert L % P == 0
    return stream_f32.reshape(L // P, P).T.copy()


def _padded_runs(run_lens_per_core: np.ndarray) -> np.ndarray:
    """[ncores, nruns] real lengths -> [nruns] padded (max over cores, ceil 128)."""
    mx = run_lens_per_core.max(axis=0)
    return ((mx + P - 1) // P) * P


def _scatter_stream(order_vals, key_sorted, run_pad_starts, run_real_starts,
                    total_len, fill, dtype):
    """Place sorted run data into a padded stream."""
    out = np.full(total_len, fill, dtype=dtype)
    n = order_vals.shape[0]
    within = np.arange(n) - run_real_starts[key_sorted]
    out[run_pad_starts[key_sorted] + within] = order_vals
    return out


def _prep(x, W1, b1, W2, b2, edge_index, edge_label_index):
    N, CIN = x.shape
    HID = W1.shape[1]
    E = edge_index.shape[1]
    NLAB = edge_label_index.shape[1]
    NS = (N + NCORES - 1) // NCORES            # real rows per shard
    NSP = ((NS + P - 1) // P) * P              # padded rows per shard
    PT = NSP // P                              # dst tiles per core
    TBL = NCORES * NSP                         # padded table rows
    NCH = (TBL + CHUNK - 1) // CHUNK
    chunk_rows = [min(CHUNK, TBL - c * CHUNK) for c in range(NCH)]
    n_batch = (PT + B_TILES - 1) // B_TILES

    src = np.asarray(edge_index[0], dtype=np.int64)
    dst = np.asarray(edge_index[1], dtype=np.int64)
    deg = np.bincount(dst, minlength=N).astype(np.float64) + 1.0
    dinv = (1.0 / np.sqrt(deg)).astype(np.float32)
    degh = np.sqrt(deg).astype(np.float32)

    # Table rows are laid out as [tblA | tblB]: tblA = first SPLIT_R rows of
    # every core's shard (= exactly chunk 0 when SPLIT_R*NCORES == CHUNK), so
    # an early small AllGather-A unblocks chunk-0 gathers while AG-B runs.
    SPLIT_R = min(CHUNK // NCORES, NSP)
    A_ROWS = SPLIT_R * NCORES

    def tbl_row(v):
        cid = np.minimum(v // NS, NCORES - 1)
        r = v - cid * NS
        return np.where(r < SPLIT_R, cid * SPLIT_R + r,
                        A_ROWS + cid * (NSP - SPLIT_R) + (r - SPLIT_R))

    # ---- aggregation streams, one per core; padded run lengths shared ----
    NRUN = PT * NCH

    def batch_key(tl, ch):
        return (tl // B_TILES) * (NCH * B_TILES) + ch * B_TILES + (tl % B_TILES)

    per_core = []
    run_lens = np.zeros((NCORES, NRUN), dtype=np.int64)
    for c in range(NCORES):
        lo, hi = c * NS, min((c + 1) * NS, N)
        m = (dst >= lo) & (dst < hi)
        es = src[m]
        edl = dst[m] - lo
        # self loops
        sl = np.arange(lo, hi, dtype=np.int64)
        es = np.concatenate([es, sl])
        edl = np.concatenate([edl, sl - lo])
        st = tbl_row(es)
        ch = st // CHUNK
        rel = (st - ch * CHUNK).astype(np.int16)
        tl = edl // P
        drel = (edl - tl * P).astype(np.float32)
        key = (tl * NCH + ch).astype(np.int64)
        order = np.argsort(batch_key(tl, ch) * 1, kind="stable")
        per_core.append((rel[order], drel[order], key[order]))
        run_lens[c] = np.bincount(key, minlength=NRUN)

    G_tc = (_padded_runs(run_lens) // P).reshape(PT, NCH)   # groups per (tile,chunk)

    # padded stream offsets in (batch, chunk, tile) order
    run_order = []   # (tile, chunk) in stream order
    for b in range(n_batch):
        tiles = list(range(b * B_TILES, min((b + 1) * B_TILES, PT)))
        for ch in range(NCH):
            for t in tiles:
                run_order.append((t, ch))
    pad_len = np.array([G_tc[t, ch] * P for (t, ch) in run_order], dtype=np.int64)
    pad_start_by_pos = np.concatenate([[0], np.cumsum(pad_len)])
    SL = int(pad_start_by_pos[-1])
    # map (tile, chunk) key -> padded start
    pos_of_key = np.zeros(NRUN, dtype=np.int64)
    for i, (t, ch) in enumerate(run_order):
        pos_of_key[t * NCH + ch] = i
    run_pad_starts = pad_start_by_pos[:-1][pos_of_key]      # by key

    eidx_tiles, edst_tiles = [], []
    for c in range(NCORES):
        rel_s, drel_s, key_s = per_core[c]
        rl = run_lens[c]
        real_starts_by_key = np.zeros(NRUN, dtype=np.int64)
        # real start of each key within the sorted-by-batchkey stream:
        order_keys = np.array([t * NCH + ch for (t, ch) in run_order])
        real_in_order = rl[order_keys]
        rs = np.concatenate([[0], np.cumsum(real_in_order)])[:-1]
        real_starts_by_key[order_keys] = rs
        eidx = _scatter_stream(rel_s, key_s, run_pad_starts, real_starts_by_key,
                               SL, 0, np.int16)
        edst = _scatter_stream(drel_s, key_s, run_pad_starts, real_starts_by_key,
                               SL, -1.0, np.float32)
        eidx_tiles.append(_pack_idx(eidx))
        edst_tiles.append(_pack_f32(edst))

    G_total = SL // P

    # group -> tile map + per-(batch,chunk) extents, in stream order
    group_tile = np.empty(G_total, dtype=np.int64)
    batches = []   # per batch: (tiles, [(chunk, g0, gcount)])
    g = 0
    for b in range(n_batch):
        tiles = list(range(b * B_TILES, min((b + 1) * B_TILES, PT)))
        runs = []
        for ch in range(NCH):
            g0 = g
            for t in tiles:
                group_tile[g:g + G_tc[t, ch]] = t
                g += int(G_tc[t, ch])
            if g > g0:
                runs.append((ch, g0, g - g0))
        batches.append((tiles, runs))
    assert g == G_total

    # ---- decode streams ----
    DNS = (NLAB + NCORES - 1) // NCORES
    NCOMBO = NCH * NCH
    dec_lens = np.zeros((NCORES, NCOMBO), dtype=np.int64)
    dec_core = []
    ls = np.asarray(edge_label_index[0], dtype=np.int64)
    ld = np.asarray(edge_label_index[1], dtype=np.int64)
    for c in range(NCORES):
        lo, hi = c * DNS, min((c + 1) * DNS, NLAB)
        lsc, ldc = tbl_row(ls[lo:hi]), tbl_row(ld[lo:hi])
        ca, cb = lsc // CHUNK, ldc // CHUNK
        combo = (ca * NCH + cb).astype(np.int64)
        order = np.argsort(combo, kind="stable")
        dec_core.append((
            (lsc - ca * CHUNK).astype(np.int16)[order],
            (ldc - cb * CHUNK).astype(np.int16)[order],
            combo[order],
            np.arange(lo, hi, dtype=np.int64)[order],   # orig pair ids
        ))
        dec_lens[c] = np.bincount(combo, minlength=NCOMBO)

    dec_pad = _padded_runs(dec_lens)                        # [NCOMBO]
    dec_starts = np.concatenate([[0], np.cumsum(dec_pad)])
    DSL = int(dec_starts[-1])
    DG_total = DSL // P

    ds_tiles, dd_tiles, dec_orig = [], [], []
    for c in range(NCORES):
        s_rel, d_rel, combo_s, orig_s = dec_core[c]
        rl = dec_lens[c]
        rs = np.concatenate([[0], np.cumsum(rl)])[:-1]
        ds = _scatter_stream(s_rel, combo_s, dec_starts[:-1], rs, DSL, 0, np.int16)
        dd = _scatter_stream(d_rel, combo_s, dec_starts[:-1], rs, DSL, 0, np.int16)
        og = _scatter_stream(orig_s, combo_s, dec_starts[:-1], rs, DSL, -1, np.int64)
        ds_tiles.append(_pack_idx(ds))
        dd_tiles.append(_pack_idx(dd))
        dec_orig.append(og)
    dec_runs = []   # (combo_src_chunk_a, combo_chunk_b, g0, gcount)
    for cmb in range(NCOMBO):
        if dec_pad[cmb]:
            dec_runs.append((cmb // NCH, cmb % NCH,
                             int(dec_starts[cmb]) // P, int(dec_pad[cmb]) // P))

    # ---- dense inputs ----
    xT = np.ascontiguousarray(np.asarray(x, dtype=np.float32).T)   # [CIN, N]
    xT_shards, dinv_tiles, degh_rows = [], [], []
    for c in range(NCORES):
        lo, hi = c * NS, min((c + 1) * NS, N)
        sh = np.zeros((CIN, NSP), dtype=np.float32)
        sh[:, : hi - lo] = xT[:, lo:hi]
        xT_shards.append(sh)
        dv = np.ones(NSP, dtype=np.float32)
        dv[: hi - lo] = dinv[lo:hi]
        dinv_tiles.append(_pack_f32(dv))                    # [128, PT]
        dg = np.zeros((1, NSP), dtype=np.float32)
        dg[0, : hi - lo] = degh[lo:hi]
        degh_rows.append(dg)

    iota = np.tile(np.arange(P, dtype=np.float32), (P, 1))
    ident = np.eye(P, dtype=np.float32)
    W1 = np.asarray(W1, dtype=np.float32)
    W2 = np.asarray(W2, dtype=np.float32)
    b1r = np.asarray(b1, dtype=np.float32).reshape(1, HID)
    b2r = np.asarray(b2, dtype=np.float32).reshape(1, HID)

    sched = dict(
        N=N, CIN=CIN, HID=HID, NS=NS, NSP=NSP, PT=PT, TBL=TBL, NCH=NCH,
        chunk_rows=chunk_rows, n_batch=n_batch, batches=batches,
        group_tile=group_tile, G_total=G_total, dec_runs=dec_runs,
        DG_total=DG_total, NLAB=NLAB, DNS=DNS, SPLIT_R=SPLIT_R, A_ROWS=A_ROWS,
    )
    inputs = [dict(
        xT=xT_shards[c], eidx=eidx_tiles[c], edst=edst_tiles[c],
        dsidx=ds_tiles[c], ddidx=dd_tiles[c], dinv_t=dinv_tiles[c],
        degh=degh_rows[c], W1=W1, W2=W2, b1r=b1r, b2r=b2r,
        iota=iota, ident=ident,
    ) for c in range(NCORES)]
    return sched, inputs, dec_orig


# ---------------------------------------------------------------- device

def _build(s):
    CIN, HID, NSP, PT, TBL, NCH = (s["CIN"], s["HID"], s["NSP"], s["PT"],
                                   s["TBL"], s["NCH"])
    G_total, DG_total = s["G_total"], s["DG_total"]
    NQ = int(os.environ.get("KNQ", "2"))
    nc = bacc.Bacc("TRN2", target_bir_lowering=False, debug=False,
                   num_devices=NCORES, num_swdge_queues=NQ)
    qctr = [0]

    def next_q():
        qctr[0] += 1
        return qctr[0] % NQ

    xT = nc.dram_tensor("xT", [CIN, NSP], F32, kind="ExternalInput")
    eidx = nc.dram_tensor("eidx", [P, G_total * 8], I16, kind="ExternalInput")
    edst = nc.dram_tensor("edst", [P, G_total], F32, kind="ExternalInput")
    dsidx = nc.dram_tensor("dsidx", [P, DG_total * 8], I16, kind="ExternalInput")
    ddidx = nc.dram_tensor("ddidx", [P, DG_total * 8], I16, kind="ExternalInput")
    dinv_t = nc.dram_tensor("dinv_t", [P, PT], F32, kind="ExternalInput")
    degh = nc.dram_tensor("degh", [1, NSP], F32, kind="ExternalInput")
    W1d = nc.dram_tensor("W1", [CIN, HID], F32, kind="ExternalInput")
    W2d = nc.dram_tensor("W2", [HID, HID], F32, kind="ExternalInput")
    b1d = nc.dram_tensor("b1r", [1, HID], F32, kind="ExternalInput")
    b2d = nc.dram_tensor("b2r", [1, HID], F32, kind="ExternalInput")
    iotad = nc.dram_tensor("iota", [P, P], F32, kind="ExternalInput")
    identd = nc.dram_tensor("ident", [P, P], F32, kind="ExternalInput")

    out = nc.dram_tensor("out", [P, DG_total], F32, kind="ExternalOutput")

    hp_sh = nc.dram_tensor("hp_sh", [NSP, HID], F32)
    h2_sh = nc.dram_tensor("h2_sh", [NSP, HID], F32)
    z_sh = nc.dram_tensor("z_sh", [NSP, HID], F32)
    SPLIT_R, A_ROWS, NSP_ = s["SPLIT_R"], s["A_ROWS"], s["NSP"]

    def mk_table(name):
        A = nc.dram_tensor(name + "A", [A_ROWS, HID], F32, addr_space="Shared")
        B = (nc.dram_tensor(name + "B", [TBL - A_ROWS, HID], F32,
                            addr_space="Shared") if TBL > A_ROWS else None)
        return (A, B)

    tbl1, tbl2, tbl3 = mk_table("tbl1"), mk_table("tbl2"), mk_table("tbl3")
    groups = [list(range(NCORES))]

    def ag_pair(sh, tabs):
        # chunk-0 rows first (small), so dependent gathers unblock early
        nc.gpsimd.collective_compute(
            "AllGather", mybir.AluOpType.bypass, replica_groups=groups,
            ins=[sh[0:SPLIT_R, :]], outs=[tabs[0][:]])
        if tabs[1] is not None:
            nc.gpsimd.collective_compute(
                "AllGather", mybir.AluOpType.bypass, replica_groups=groups,
                ins=[sh[SPLIT_R:NSP_, :]], outs=[tabs[1][:]])

    def chunk_ap(tabs, ch, rows):
        if ch == 0:
            return tabs[0][0:rows, :]
        base = sum(s["chunk_rows"][:ch]) - A_ROWS
        return tabs[1][base:base + rows, :]

    KT = CIN // P   # k chunks for layer-1 matmul

    with tile.TileContext(nc) as tc:
        with ExitStack() as root:
            cp = root.enter_context(tc.tile_pool(name="const", bufs=1))
            W1_sb = cp.tile([P, KT * HID], F32)
            for k in range(KT):
                nc.sync.dma_start(W1_sb[:, k * HID:(k + 1) * HID],
                                  W1d[k * P:(k + 1) * P, :])
            W2_sb = cp.tile([P, HID], F32)
            nc.sync.dma_start(W2_sb[:], W2d[:])
            b1_sb = cp.tile([1, HID], F32)
            nc.sync.dma_start(b1_sb[:], b1d[:])
            b2_sb = cp.tile([1, HID], F32)
            nc.sync.dma_start(b2_sb[:], b2d[:])
            iota_sb = cp.tile([P, P], F32)
            nc.sync.dma_start(iota_sb[:], iotad[:])
            ident_sb = cp.tile([P, P], F32)
            nc.sync.dma_start(ident_sb[:], identd[:])
            dinv_sb = cp.tile([P, PT], F32)
            nc.sync.dma_start(dinv_sb[:], dinv_t[:])
            degh_sb = cp.tile([1, NSP], F32)
            nc.sync.dma_start(degh_sb[:], degh[:])
            eidx_sb = cp.tile([P, G_total * 8], I16)
            nc.sync.dma_start(eidx_sb[:], eidx[:])
            edst_sb = cp.tile([P, G_total], F32)
            nc.sync.dma_start(edst_sb[:], edst[:])

            # ---------------- phase 1: hp = (x @ W1) * dinv ----------------
            with ExitStack() as ph:
                xp = ph.enter_context(tc.tile_pool(name="xp", bufs=3))
                op = ph.enter_context(tc.tile_pool(name="op", bufs=2))
                pp = ph.enter_context(tc.tile_pool(name="pp", bufs=4,
                                                   space="PSUM"))
                OB = 4   # tiles per input/output DMA batch
                for blk in range(0, PT, OB):
                    nt = min(OB, PT - blk)
                    ob = op.tile([P, OB * HID], F32, tag="hpout")
                    xt = xp.tile([P, KT, OB * P], F32, tag="xt")
                    for k in range(KT):
                        nc.sync.dma_start(
                            xt[:, k, :nt * P],
                            xT[k * P:(k + 1) * P, blk * P:(blk + nt) * P])
                    for j in range(nt):
                        t = blk + j
                        ps = pp.tile([P, HID], F32, tag="p1")
                        for k in range(KT):
                            nc.tensor.matmul(
                                out=ps[:], lhsT=xt[:, k, j * P:(j + 1) * P],
                                rhs=W1_sb[:, k * HID:(k + 1) * HID],
                                start=(k == 0), stop=(k == KT - 1))
                        nc.scalar.activation(
                            ob[:, j * HID:(j + 1) * HID], ps[:],
                            mybir.ActivationFunctionType.Identity,
                            scale=dinv_sb[:, t:t + 1])
                    dr = _rows_ap(hp_sh, blk * P, nt, HID)
                    nc.sync.dma_start(dr, ob[:, :nt * HID].rearrange(
                        "p (b f) -> p b f", b=nt))

            ag_pair(hp_sh, tbl1)

            PH = int(os.environ.get("KPHASE", "5"))

            AGGMODE = os.environ.get("AGGMODE", "full")

            # ---------------- aggregation layers ----------------
            def agg_layer(table, brow, relu, l2_tail, out_sh):
                with ExitStack() as ph:
                    mp = ph.enter_context(tc.tile_pool(name="mp", bufs=10))
                    sp = ph.enter_context(tc.tile_pool(name="sp", bufs=4))
                    zp = ph.enter_context(tc.tile_pool(name="zp", bufs=2))
                    ap = ph.enter_context(tc.tile_pool(name="ap", bufs=1,
                                                       space="PSUM"))
                    for tiles, runs in s["batches"]:
                        psums = {}
                        started = set()
                        for (ch, g0, gcount) in runs:
                            rows = s["chunk_rows"][ch]
                            for p0 in range(0, gcount, RING_G):
                                pc = min(RING_G, gcount - p0)
                                gg = g0 + p0
                                m = mp.tile([P, RING_G, P], F32, tag="msg")
                                if "nogather" in AGGMODE:
                                    nc.vector.memset(m[:, :pc, :], 0.0)
                                else:
                                    nc.gpsimd.dma_gather(
                                        m[:, :pc, :], chunk_ap(table, ch, rows),
                                        eidx_sb[:, gg * 8:(gg + pc) * 8],
                                        num_idxs=pc * P, num_idxs_reg=pc * P,
                                        elem_size=HID, queue_num=next_q())
                                if "nosel" not in AGGMODE:
                                    # all pc select matrices in one DVE op:
                                    # sel[p,q,j] = (iota[p,j] == edst[p,gg+q])
                                    selb = sp.tile([P, RING_G, P], F32,
                                                   tag="sel")
                                    ia = iota_sb[:]
                                    iota_b = bass.AP(
                                        tensor=ia.tensor, offset=ia.offset,
                                        ap=[list(ia.ap[0]), [0, pc],
                                            list(ia.ap[1])])
                                    ea = edst_sb[:, gg:gg + pc]
                                    edst_b = bass.AP(
                                        tensor=ea.tensor, offset=ea.offset,
                                        ap=[list(ea.ap[0]), list(ea.ap[1]),
                                            [0, P]])
                                    nc.vector.tensor_tensor(
                                        selb[:, :pc, :], iota_b, edst_b,
                                        op=mybir.AluOpType.is_equal)
                                for q in range(pc):
                                    gq = gg + q
                                    t = int(s["group_tile"][gq])
                                    if t not in psums:
                                        psums[t] = ap.tile(
                                            [P, HID], F32, name=f"psum{t}",
                                            tag=f"acc{t % B_TILES}")
                                    if "nosel" in AGGMODE:
                                        continue
                                    nc.tensor.matmul(
                                        out=psums[t][:], lhsT=selb[:, q, :],
                                        rhs=m[:, q, :],
                                        start=(t not in started), stop=False,
                                        skip_group_check=True)
                                    started.add(t)
                        ob = zp.tile([P, B_TILES * HID], F32, tag="zout")
                        for j, t in enumerate(tiles):
                            if "nobias" not in AGGMODE:
                                nc.tensor.matmul(
                                    out=psums[t][:],
                                    lhsT=degh_sb[:1, t * P:(t + 1) * P],
                                    rhs=brow[:1, :],
                                    start=("nosel" in AGGMODE),
                                    stop=True, skip_group_check=True)
                            zt = ob[:, j * HID:(j + 1) * HID]
                            if "noact" in AGGMODE:
                                nc.vector.memset(zt, 0.0)
                            else:
                                nc.scalar.activation(
                                    zt, psums[t][:],
                                    mybir.ActivationFunctionType.Relu if relu
                                    else mybir.ActivationFunctionType.Identity,
                                    scale=dinv_sb[:, t:t + 1])
                            if l2_tail and "notail" not in AGGMODE:
                                trp = ap.tile([P, P], F32, tag="tr")
                                nc.tensor.transpose(trp[:], zt, ident_sb[:])
                                a1t = sp.tile([P, P], F32, tag="a1t")
                                nc.vector.tensor_copy(a1t[:], trp[:])
                                h2p = ap.tile([P, HID], F32, tag="h2")
                                nc.tensor.matmul(out=h2p[:], lhsT=a1t[:],
                                                 rhs=W2_sb[:], start=True,
                                                 stop=True)
                                h2s = sp.tile([P, HID], F32, tag="h2s")
                                nc.scalar.activation(
                                    h2s[:], h2p[:],
                                    mybir.ActivationFunctionType.Identity,
                                    scale=dinv_sb[:, t:t + 1])
                                nc.sync.dma_start(
                                    out_sh[t * P:(t + 1) * P, :], h2s[:])
                        if not l2_tail:
                            nt = len(tiles)
                            dr = _rows_ap(out_sh, tiles[0] * P, nt, HID)
                            nc.sync.dma_start(dr, ob[:, :nt * HID].rearrange(
                                "p (b f) -> p b f", b=nt))

            if PH >= 2:
                agg_layer(tbl1, b1_sb, relu=True, l2_tail=True, out_sh=h2_sh)
            if PH >= 3:
                ag_pair(h2_sh, tbl2)
                agg_layer(tbl2, b2_sb, relu=False, l2_tail=False, out_sh=z_sh)
            if PH >= 4:
                ag_pair(z_sh, tbl3)

            if PH < 5:
                with ExitStack() as ph:
                    zp0 = ph.enter_context(tc.tile_pool(name="zp0", bufs=1))
                    oz = zp0.tile([P, DG_total], F32)
                    nc.vector.memset(oz[:], 0.0)
                    nc.sync.dma_start(out[:], oz[:])

            # ---------------- decode ----------------
            if PH >= 5:
              with ExitStack() as ph:
                dp = ph.enter_context(tc.tile_pool(name="dp", bufs=4))
                dip = ph.enter_context(tc.tile_pool(name="dip", bufs=1))
                ds_sb = dip.tile([P, DG_total * 8], I16)
                nc.sync.dma_start(ds_sb[:], dsidx[:])
                dd_sb = dip.tile([P, DG_total * 8], I16)
                nc.sync.dma_start(dd_sb[:], ddidx[:])
                oacc = dip.tile([P, DG_total], F32)
                for (ca, cb, g0, gcount) in s["dec_runs"]:
                    for p0 in range(0, gcount, DEC_RING):
                        pc = min(DEC_RING, gcount - p0)
                        gg = g0 + p0
                        ms = dp.tile([P, DEC_RING, P], F32, tag="ds")
                        nc.gpsimd.dma_gather(
                            ms[:, :pc, :],
                            chunk_ap(tbl3, ca, s["chunk_rows"][ca]),
                            ds_sb[:, gg * 8:(gg + pc) * 8],
                            num_idxs=pc * P, num_idxs_reg=pc * P,
                            elem_size=HID, queue_num=next_q())
                        md = dp.tile([P, DEC_RING, P], F32, tag="dd")
                        nc.gpsimd.dma_gather(
                            md[:, :pc, :],
                            chunk_ap(tbl3, cb, s["chunk_rows"][cb]),
                            dd_sb[:, gg * 8:(gg + pc) * 8],
                            num_idxs=pc * P, num_idxs_reg=pc * P,
                            elem_size=HID, queue_num=next_q())
                        pr = dp.tile([P, DEC_RING, P], F32, tag="pr")
                        nc.vector.tensor_tensor(pr[:, :pc, :], ms[:, :pc, :],
                                                md[:, :pc, :],
                                                op=mybir.AluOpType.mult)
                        nc.vector.tensor_reduce(
                            oacc[:, gg:gg + pc], pr[:, :pc, :],
                            axis=mybir.AxisListType.X, op=mybir.AluOpType.add)
                nc.sync.dma_start(out[:], oacc[:])

    nc.compile()
    return nc


# ---------------------------------------------------------------- entry

def kernel(x, W1, b1, W2, b2, edge_index, edge_label_index):
    x = np.asarray(x)
    sched, in_maps, dec_orig = _prep(x, W1, b1, W2, b2,
                                     edge_index, edge_label_index)
    nc = _build(sched)
    res = run_bass_kernel_spmd(nc, in_maps, core_ids=list(range(NCORES)))
    NLAB = sched["NLAB"]
    outf = np.zeros(NLAB, dtype=np.float32)
    for c in range(NCORES):
        vals = res.results[c]["out"].T.ravel()       # stream order
        og = dec_orig[c]
        mreal = og >= 0
        outf[og[mreal]] = vals[mreal]
    return outf



# revision 5
# speedup vs baseline: 1.0105x; 1.0105x over previous
"""GCN link predictor on 8 trn2 NeuronCores (Bass/Tile).

Sharding: nodes row-sharded across 8 cores (dst-partitioned edges so the
segment-sum stays local); per-layer full feature tables shared via AllGather;
decode edges sharded data-parallel.

Math: gcn_conv(x) = D^-1/2 (A+I) D^-1/2 (x W) + b.  We fold D^-1/2 into the
tables: h' = (x W) * dinv[row], out[d] = dinv[d] * sum_{s in N(d)} h'[s] + b.
The per-edge norm disappears; aggregation is a 0/1 select-matrix matmul on the
TensorEngine over edges gathered by dma_gather (512B rows at HBM line rate).
The +b is a rank-1 matmul (sqrt(deg)[d] x b) folded into the same PSUM
accumulation, so the epilogue is one ScalarE activation (scale=dinv, Relu/Id).
"""
import math
import os
import numpy as np
from contextlib import ExitStack

import concourse.bass as bass
import concourse.bacc as bacc
import concourse.tile as tile
import concourse.mybir as mybir
from concourse.bass_utils import run_bass_kernel_spmd

F32 = mybir.dt.float32
I16 = mybir.dt.int16


def _rows_ap(handle, row0: int, nblk: int, width: int):
    """DRAM [rows, width] view as [128, nblk, width]: p,b,f -> row row0+b*128+p."""
    a = handle[:]
    return bass.AP(tensor=a.tensor, offset=row0 * width,
                   ap=[[width, 128], [128 * width, nblk], [1, width]])
NCORES = 8
P = 128
CHUNK = 32768          # int16-addressable rows per gather table chunk
B_TILES = 6            # dst tiles per aggregation batch (PSUM banks: 6+1+1)
RING_G = int(os.environ.get("KRING", "4"))
                       # msg ring slot size in 128-edge groups (HW: >1024 idxs
                       # per dma_gather faults; 512 adds margin vs flakiness)
DEC_RING = int(os.environ.get("KDRING", "4"))  # decode ring slot size in groups


# ---------------------------------------------------------------- host prep

def _pack_idx(stream_i16: np.ndarray) -> np.ndarray:
    """int16 stream -> [128, L/16] tile (pos i -> [i%16, i//16], x8 replicated)."""
    L = stream_i16.shape[0]
    assert L % 16 == 0
    a16 = stream_i16.reshape(L // 16, 16).T.copy()
    return np.tile(a16, (8, 1))


def _pack_f32(stream_f32: np.ndarray) -> np.ndarray:
    """f32 stream -> [128, L/128] tile (pos i -> [i%128, i//128])."""
    L = stream_f32.shape[0]
    assert L % P == 0
    return stream_f32.reshape(L // P, P).T.copy()


def _padded_runs(run_lens_per_core: np.ndarray) -> np.ndarray:
    """[ncores, nruns] real lengths -> [nruns] padded (max over cores, ceil 128)."""
    mx = run_lens_per_core.max(axis=0)
    return ((mx + P - 1) // P) * P


def _scatter_stream(order_vals, key_sorted, run_pad_starts, run_real_starts,
                    total_len, fill, dtype):
    """Place sorted run data into a padded stream."""
    out = np.full(total_len, fill, dtype=dtype)
    n = order_vals.shape[0]
    within = np.arange(n) - run_real_starts[key_sorted]
    out[run_pad_starts[key_sorted] + within] = order_vals
    return out


def _prep(x, W1, b1, W2, b2, edge_index, edge_label_index):
    N, CIN = x.shape
    HID = W1.shape[1]
    E = edge_index.shape[1]
    NLAB = edge_label_index.shape[1]
    NS = (N + NCORES - 1) // NCORES            # real rows per shard
    NSP = ((NS + P - 1) // P) * P              # padded rows per shard
    PT = NSP // P                              # dst tiles per core
    TBL = NCORES * NSP                         # padded table rows
    NCH = (TBL + CHUNK - 1) // CHUNK
    chunk_rows = [min(CHUNK, TBL - c * CHUNK) for c in range(NCH)]
    n_batch = (PT + B_TILES - 1) // B_TILES

    src = np.asarray(edge_index[0], dtype=np.int64)
    dst = np.asarray(edge_index[1], dtype=np.int64)
    deg = np.bincount(dst, minlength=N).astype(np.float64) + 1.0
    dinv = (1.0 / np.sqrt(deg)).astype(np.float32)
    degh = np.sqrt(deg).astype(np.float32)

    # Table rows are laid out as [tblA | tblB]: tblA = first SPLIT_R rows of
    # every core's shard (= exactly chunk 0 when SPLIT_R*NCORES == CHUNK), so
    # an early small AllGather-A unblocks chunk-0 gathers while AG-B runs.
    SPLIT_R = min(CHUNK // NCORES, NSP)
    A_ROWS = SPLIT_R * NCORES

    def tbl_row(v):
        cid = np.minimum(v // NS, NCORES - 1)
        r = v - cid * NS
        return np.where(r < SPLIT_R, cid * SPLIT_R + r,
                        A_ROWS + cid * (NSP - SPLIT_R) + (r - SPLIT_R))

    # ---- aggregation streams, one per core; padded run lengths shared ----
    NRUN = PT * NCH

    def batch_key(tl, ch):
        return (tl // B_TILES) * (NCH * B_TILES) + ch * B_TILES + (tl % B_TILES)

    per_core = []
    run_lens = np.zeros((NCORES, NRUN), dtype=np.int64)
    for c in range(NCORES):
        lo, hi = c * NS, min((c + 1) * NS, N)
        m = (dst >= lo) & (dst < hi)
        es = src[m]
        edl = dst[m] - lo
        # self loops
        sl = np.arange(lo, hi, dtype=np.int64)
        es = np.concatenate([es, sl])
        edl = np.concatenate([edl, sl - lo])
        st = tbl_row(es)
        ch = st // CHUNK
        rel = (st - ch * CHUNK).astype(np.int16)
        tl = edl // P
        drel = (edl - tl * P).astype(np.float32)
        key = (tl * NCH + ch).astype(np.int64)
        order = np.argsort(batch_key(tl, ch) * 1, kind="stable")
        per_core.append((rel[order], drel[order], key[order]))
        run_lens[c] = np.bincount(key, minlength=NRUN)

    G_tc = (_padded_runs(run_lens) // P).reshape(PT, NCH)   # groups per (tile,chunk)

    # padded stream offsets in (batch, chunk, tile) order
    run_order = []   # (tile, chunk) in stream order
    for b in range(n_batch):
        tiles = list(range(b * B_TILES, min((b + 1) * B_TILES, PT)))
        for ch in range(NCH):
            for t in tiles:
                run_order.append((t, ch))
    pad_len = np.array([G_tc[t, ch] * P for (t, ch) in run_order], dtype=np.int64)
    pad_start_by_pos = np.concatenate([[0], np.cumsum(pad_len)])
    SL = int(pad_start_by_pos[-1])
    # map (tile, chunk) key -> padded start
    pos_of_key = np.zeros(NRUN, dtype=np.int64)
    for i, (t, ch) in enumerate(run_order):
        pos_of_key[t * NCH + ch] = i
    run_pad_starts = pad_start_by_pos[:-1][pos_of_key]      # by key

    eidx_tiles, edst_tiles = [], []
    for c in range(NCORES):
        rel_s, drel_s, key_s = per_core[c]
        rl = run_lens[c]
        real_starts_by_key = np.zeros(NRUN, dtype=np.int64)
        # real start of each key within the sorted-by-batchkey stream:
        order_keys = np.array([t * NCH + ch for (t, ch) in run_order])
        real_in_order = rl[order_keys]
        rs = np.concatenate([[0], np.cumsum(real_in_order)])[:-1]
        real_starts_by_key[order_keys] = rs
        eidx = _scatter_stream(rel_s, key_s, run_pad_starts, real_starts_by_key,
                               SL, 0, np.int16)
        edst = _scatter_stream(drel_s, key_s, run_pad_starts, real_starts_by_key,
                               SL, -1.0, np.float32)
        eidx_tiles.append(_pack_idx(eidx))
        edst_tiles.append(_pack_f32(edst))

    G_total = SL // P

    # group -> tile map + per-(batch,chunk) extents, in stream order
    group_tile = np.empty(G_total, dtype=np.int64)
    batches = []   # per batch: (tiles, [(chunk, g0, gcount)])
    g = 0
    for b in range(n_batch):
        tiles = list(range(b * B_TILES, min((b + 1) * B_TILES, PT)))
        runs = []
        for ch in range(NCH):
            g0 = g
            for t in tiles:
                group_tile[g:g + G_tc[t, ch]] = t
                g += int(G_tc[t, ch])
            if g > g0:
                runs.append((ch, g0, g - g0))
        batches.append((tiles, runs))
    assert g == G_total

    # ---- decode streams ----
    DNS = (NLAB + NCORES - 1) // NCORES
    NCOMBO = NCH * NCH
    dec_lens = np.zeros((NCORES, NCOMBO), dtype=np.int64)
    dec_core = []
    ls = np.asarray(edge_label_index[0], dtype=np.int64)
    ld = np.asarray(edge_label_index[1], dtype=np.int64)
    for c in range(NCORES):
        lo, hi = c * DNS, min((c + 1) * DNS, NLAB)
        lsc, ldc = tbl_row(ls[lo:hi]), tbl_row(ld[lo:hi])
        ca, cb = lsc // CHUNK, ldc // CHUNK
        combo = (ca * NCH + cb).astype(np.int64)
        order = np.argsort(combo, kind="stable")
        dec_core.append((
            (lsc - ca * CHUNK).astype(np.int16)[order],
            (ldc - cb * CHUNK).astype(np.int16)[order],
            combo[order],
            np.arange(lo, hi, dtype=np.int64)[order],   # orig pair ids
        ))
        dec_lens[c] = np.bincount(combo, minlength=NCOMBO)

    dec_pad = _padded_runs(dec_lens)                        # [NCOMBO]
    dec_starts = np.concatenate([[0], np.cumsum(dec_pad)])
    DSL = int(dec_starts[-1])
    DG_total = DSL // P

    ds_tiles, dd_tiles, dec_orig = [], [], []
    for c in range(NCORES):
        s_rel, d_rel, combo_s, orig_s = dec_core[c]
        rl = dec_lens[c]
        rs = np.concatenate([[0], np.cumsum(rl)])[:-1]
        ds = _scatter_stream(s_rel, combo_s, dec_starts[:-1], rs, DSL, 0, np.int16)
        dd = _scatter_stream(d_rel, combo_s, dec_starts[:-1], rs, DSL, 0, np.int16)
        og = _scatter_stream(orig_s, combo_s, dec_starts[:-1], rs, DSL, -1, np.int64)
        ds_tiles.append(_pack_idx(ds))
        dd_tiles.append(_pack_idx(dd))
        dec_orig.append(og)
    dec_runs = []   # (combo_src_chunk_a, combo_chunk_b, g0, gcount)
    for cmb in range(NCOMBO):
        if dec_pad[cmb]:
            dec_runs.append((cmb // NCH, cmb % NCH,
                             int(dec_starts[cmb]) // P, int(dec_pad[cmb]) // P))

    # ---- dense inputs ----
    xT = np.ascontiguousarray(np.asarray(x, dtype=np.float32).T)   # [CIN, N]
    xT_shards, dinv_tiles, degh_rows = [], [], []
    for c in range(NCORES):
        lo, hi = c * NS, min((c + 1) * NS, N)
        sh = np.zeros((CIN, NSP), dtype=np.float32)
        sh[:, : hi - lo] = xT[:, lo:hi]
        xT_shards.append(sh)
        dv = np.ones(NSP, dtype=np.float32)
        dv[: hi - lo] = dinv[lo:hi]
        dinv_tiles.append(_pack_f32(dv))                    # [128, PT]
        dg = np.zeros((1, NSP), dtype=np.float32)
        dg[0, : hi - lo] = degh[lo:hi]
        degh_rows.append(dg)

    iota = np.tile(np.arange(P, dtype=np.float32), (P, 1))
    ident = np.eye(P, dtype=np.float32)
    W1 = np.asarray(W1, dtype=np.float32)
    W2 = np.asarray(W2, dtype=np.float32)
    b1r = np.asarray(b1, dtype=np.float32).reshape(1, HID)
    b2r = np.asarray(b2, dtype=np.float32).reshape(1, HID)

    sched = dict(
        N=N, CIN=CIN, HID=HID, NS=NS, NSP=NSP, PT=PT, TBL=TBL, NCH=NCH,
        chunk_rows=chunk_rows, n_batch=n_batch, batches=batches,
        group_tile=group_tile, G_total=G_total, dec_runs=dec_runs,
        DG_total=DG_total, NLAB=NLAB, DNS=DNS, SPLIT_R=SPLIT_R, A_ROWS=A_ROWS,
    )
    inputs = [dict(
        xT=xT_shards[c], eidx=eidx_tiles[c], edst=edst_tiles[c],
        dsidx=ds_tiles[c], ddidx=dd_tiles[c], dinv_t=dinv_tiles[c],
        degh=degh_rows[c], W1=W1, W2=W2, b1r=b1r, b2r=b2r,
        iota=iota, ident=ident,
    ) for c in range(NCORES)]
    return sched, inputs, dec_orig


# ---------------------------------------------------------------- device

def _build(s):
    CIN, HID, NSP, PT, TBL, NCH = (s["CIN"], s["HID"], s["NSP"], s["PT"],
                                   s["TBL"], s["NCH"])
    G_total, DG_total = s["G_total"], s["DG_total"]
    NQ = int(os.environ.get("KNQ", "2"))
    KSIM = os.environ.get("KSIM", "") == "1"   # single-core collective-free
    KSCRATCH = int(os.environ.get("KSCRATCH", "16384"))
    nc = bacc.Bacc("TRN2", target_bir_lowering=False, debug=False,
                   num_devices=1 if KSIM else NCORES, num_swdge_queues=NQ,
                   dynamic_dma_scratch_size=KSCRATCH)
    qctr = [0]

    def next_q():
        qctr[0] += 1
        return qctr[0] % NQ

    xT = nc.dram_tensor("xT", [CIN, NSP], F32, kind="ExternalInput")
    eidx = nc.dram_tensor("eidx", [P, G_total * 8], I16, kind="ExternalInput")
    edst = nc.dram_tensor("edst", [P, G_total], F32, kind="ExternalInput")
    dsidx = nc.dram_tensor("dsidx", [P, DG_total * 8], I16, kind="ExternalInput")
    ddidx = nc.dram_tensor("ddidx", [P, DG_total * 8], I16, kind="ExternalInput")
    dinv_t = nc.dram_tensor("dinv_t", [P, PT], F32, kind="ExternalInput")
    degh = nc.dram_tensor("degh", [1, NSP], F32, kind="ExternalInput")
    W1d = nc.dram_tensor("W1", [CIN, HID], F32, kind="ExternalInput")
    W2d = nc.dram_tensor("W2", [HID, HID], F32, kind="ExternalInput")
    b1d = nc.dram_tensor("b1r", [1, HID], F32, kind="ExternalInput")
    b2d = nc.dram_tensor("b2r", [1, HID], F32, kind="ExternalInput")
    iotad = nc.dram_tensor("iota", [P, P], F32, kind="ExternalInput")
    identd = nc.dram_tensor("ident", [P, P], F32, kind="ExternalInput")

    out = nc.dram_tensor("out", [P, DG_total], F32, kind="ExternalOutput")

    hp_sh = nc.dram_tensor("hp_sh", [NSP, HID], F32)
    h2_sh = nc.dram_tensor("h2_sh", [NSP, HID], F32)
    z_sh = nc.dram_tensor("z_sh", [NSP, HID], F32)
    SPLIT_R, A_ROWS, NSP_ = s["SPLIT_R"], s["A_ROWS"], s["NSP"]

    def mk_table(name):
        A = nc.dram_tensor(name + "A", [A_ROWS, HID], F32, addr_space="Shared")
        B = (nc.dram_tensor(name + "B", [TBL - A_ROWS, HID], F32,
                            addr_space="Shared") if TBL > A_ROWS else None)
        return (A, B)

    tbl1, tbl2, tbl3 = mk_table("tbl1"), mk_table("tbl2"), mk_table("tbl3")
    groups = [list(range(NCORES))]

    def ag_pair(sh, tabs):
        if KSIM:
            # sim stand-in: write the same local bytes the AllGather would
            for c in range(NCORES):
                nc.sync.dma_start(
                    tabs[0][c * SPLIT_R:(c + 1) * SPLIT_R, :],
                    sh[0:SPLIT_R, :])
                if tabs[1] is not None:
                    rb = NSP_ - SPLIT_R
                    nc.sync.dma_start(
                        tabs[1][c * rb:(c + 1) * rb, :],
                        sh[SPLIT_R:NSP_, :])
            return
        # chunk-0 rows first (small), so dependent gathers unblock early
        nc.gpsimd.collective_compute(
            "AllGather", mybir.AluOpType.bypass, replica_groups=groups,
            ins=[sh[0:SPLIT_R, :]], outs=[tabs[0][:]])
        if tabs[1] is not None:
            nc.gpsimd.collective_compute(
                "AllGather", mybir.AluOpType.bypass, replica_groups=groups,
                ins=[sh[SPLIT_R:NSP_, :]], outs=[tabs[1][:]])

    def chunk_ap(tabs, ch, rows):
        if ch == 0:
            return tabs[0][0:rows, :]
        base = sum(s["chunk_rows"][:ch]) - A_ROWS
        return tabs[1][base:base + rows, :]

    KT = CIN // P   # k chunks for layer-1 matmul

    with tile.TileContext(nc) as tc:
        with ExitStack() as root:
            cp = root.enter_context(tc.tile_pool(name="const", bufs=1))
            W1_sb = cp.tile([P, KT * HID], F32)
            for k in range(KT):
                nc.sync.dma_start(W1_sb[:, k * HID:(k + 1) * HID],
                                  W1d[k * P:(k + 1) * P, :])
            W2_sb = cp.tile([P, HID], F32)
            nc.sync.dma_start(W2_sb[:], W2d[:])
            b1_sb = cp.tile([1, HID], F32)
            nc.sync.dma_start(b1_sb[:], b1d[:])
            b2_sb = cp.tile([1, HID], F32)
            nc.sync.dma_start(b2_sb[:], b2d[:])
            iota_sb = cp.tile([P, P], F32)
            nc.sync.dma_start(iota_sb[:], iotad[:])
            ident_sb = cp.tile([P, P], F32)
            nc.sync.dma_start(ident_sb[:], identd[:])
            dinv_sb = cp.tile([P, PT], F32)
            nc.sync.dma_start(dinv_sb[:], dinv_t[:])
            degh_sb = cp.tile([1, NSP], F32)
            nc.sync.dma_start(degh_sb[:], degh[:])
            eidx_sb = cp.tile([P, G_total * 8], I16)
            nc.sync.dma_start(eidx_sb[:], eidx[:])
            edst_sb = cp.tile([P, G_total], F32)
            nc.sync.dma_start(edst_sb[:], edst[:])

            # ---------------- phase 1: hp = (x @ W1) * dinv ----------------
            with ExitStack() as ph:
                xp = ph.enter_context(tc.tile_pool(name="xp", bufs=3))
                op = ph.enter_context(tc.tile_pool(name="op", bufs=2))
                pp = ph.enter_context(tc.tile_pool(name="pp", bufs=4,
                                                   space="PSUM"))
                OB = 4   # tiles per input/output DMA batch
                for blk in range(0, PT, OB):
                    nt = min(OB, PT - blk)
                    ob = op.tile([P, OB * HID], F32, tag="hpout")
                    xt = xp.tile([P, KT, OB * P], F32, tag="xt")
                    for k in range(KT):
                        nc.sync.dma_start(
                            xt[:, k, :nt * P],
                            xT[k * P:(k + 1) * P, blk * P:(blk + nt) * P])
                    for j in range(nt):
                        t = blk + j
                        ps = pp.tile([P, HID], F32, tag="p1")
                        for k in range(KT):
                            nc.tensor.matmul(
                                out=ps[:], lhsT=xt[:, k, j * P:(j + 1) * P],
                                rhs=W1_sb[:, k * HID:(k + 1) * HID],
                                start=(k == 0), stop=(k == KT - 1))
                        nc.scalar.activation(
                            ob[:, j * HID:(j + 1) * HID], ps[:],
                            mybir.ActivationFunctionType.Identity,
                            scale=dinv_sb[:, t:t + 1])
                    dr = _rows_ap(hp_sh, blk * P, nt, HID)
                    nc.sync.dma_start(dr, ob[:, :nt * HID].rearrange(
                        "p (b f) -> p b f", b=nt))

            ag_pair(hp_sh, tbl1)

            PH = int(os.environ.get("KPHASE", "5"))

            AGGMODE = os.environ.get("AGGMODE", "full")

            # ---------------- aggregation layers ----------------
            def agg_layer(table, brow, relu, l2_tail, out_sh):
                with ExitStack() as ph:
                    mp = ph.enter_context(tc.tile_pool(name="mp", bufs=10))
                    sp = ph.enter_context(tc.tile_pool(name="sp", bufs=4))
                    zp = ph.enter_context(tc.tile_pool(name="zp", bufs=2))
                    ap = ph.enter_context(tc.tile_pool(name="ap", bufs=1,
                                                       space="PSUM"))
                    for tiles, runs in s["batches"]:
                        psums = {}
                        started = set()
                        for (ch, g0, gcount) in runs:
                            rows = s["chunk_rows"][ch]
                            for p0 in range(0, gcount, RING_G):
                                pc = min(RING_G, gcount - p0)
                                gg = g0 + p0
                                m = mp.tile([P, RING_G, P], F32, tag="msg")
                                if "nogather" in AGGMODE:
                                    nc.vector.memset(m[:, :pc, :], 0.0)
                                else:
                                    nc.gpsimd.dma_gather(
                                        m[:, :pc, :], chunk_ap(table, ch, rows),
                                        eidx_sb[:, gg * 8:(gg + pc) * 8],
                                        num_idxs=pc * P, num_idxs_reg=pc * P,
                                        elem_size=HID, queue_num=next_q())
                                if "nosel" not in AGGMODE:
                                    # all pc select matrices in one DVE op:
                                    # sel[p,q,j] = (iota[p,j] == edst[p,gg+q])
                                    selb = sp.tile([P, RING_G, P], F32,
                                                   tag="sel")
                                    ia = iota_sb[:]
                                    iota_b = bass.AP(
                                        tensor=ia.tensor, offset=ia.offset,
                                        ap=[list(ia.ap[0]), [0, pc],
                                            list(ia.ap[1])])
                                    ea = edst_sb[:, gg:gg + pc]
                                    edst_b = bass.AP(
                                        tensor=ea.tensor, offset=ea.offset,
                                        ap=[list(ea.ap[0]), list(ea.ap[1]),
                                            [0, P]])
                                    nc.vector.tensor_tensor(
                                        selb[:, :pc, :], iota_b, edst_b,
                                        op=mybir.AluOpType.is_equal)
                                for q in range(pc):
                                    gq = gg + q
                                    t = int(s["group_tile"][gq])
                                    if t not in psums:
                                        psums[t] = ap.tile(
                                            [P, HID], F32, name=f"psum{t}",
                                            tag=f"acc{t % B_TILES}")
                                    if "nosel" in AGGMODE:
                                        continue
                                    nc.tensor.matmul(
                                        out=psums[t][:], lhsT=selb[:, q, :],
                                        rhs=m[:, q, :],
                                        start=(t not in started), stop=False,
                                        skip_group_check=True)
                                    started.add(t)
                        ob = zp.tile([P, B_TILES * HID], F32, tag="zout")
                        for j, t in enumerate(tiles):
                            if "nobias" not in AGGMODE:
                                nc.tensor.matmul(
                                    out=psums[t][:],
                                    lhsT=degh_sb[:1, t * P:(t + 1) * P],
                                    rhs=brow[:1, :],
                                    start=("nosel" in AGGMODE),
                                    stop=True, skip_group_check=True)
                            zt = ob[:, j * HID:(j + 1) * HID]
                            if "noact" in AGGMODE:
                                nc.vector.memset(zt, 0.0)
                            else:
                                nc.scalar.activation(
                                    zt, psums[t][:],
                                    mybir.ActivationFunctionType.Relu if relu
                                    else mybir.ActivationFunctionType.Identity,
                                    scale=dinv_sb[:, t:t + 1])
                            if l2_tail and "notail" not in AGGMODE:
                                trp = ap.tile([P, P], F32, tag="tr")
                                nc.tensor.transpose(trp[:], zt, ident_sb[:])
                                a1t = sp.tile([P, P], F32, tag="a1t")
                                nc.vector.tensor_copy(a1t[:], trp[:])
                                h2p = ap.tile([P, HID], F32, tag="h2")
                                nc.tensor.matmul(out=h2p[:], lhsT=a1t[:],
                                                 rhs=W2_sb[:], start=True,
                                                 stop=True)
                                h2s = sp.tile([P, HID], F32, tag="h2s")
                                nc.scalar.activation(
                                    h2s[:], h2p[:],
                                    mybir.ActivationFunctionType.Identity,
                                    scale=dinv_sb[:, t:t + 1])
                                nc.sync.dma_start(
                                    out_sh[t * P:(t + 1) * P, :], h2s[:])
                        if not l2_tail:
                            nt = len(tiles)
                            dr = _rows_ap(out_sh, tiles[0] * P, nt, HID)
                            nc.sync.dma_start(dr, ob[:, :nt * HID].rearrange(
                                "p (b f) -> p b f", b=nt))

            if PH >= 2:
                agg_layer(tbl1, b1_sb, relu=True, l2_tail=True, out_sh=h2_sh)
            if PH >= 3:
                ag_pair(h2_sh, tbl2)
                agg_layer(tbl2, b2_sb, relu=False, l2_tail=False, out_sh=z_sh)
            if PH >= 4:
                ag_pair(z_sh, tbl3)

            if PH < 5:
                with ExitStack() as ph:
                    zp0 = ph.enter_context(tc.tile_pool(name="zp0", bufs=1))
                    oz = zp0.tile([P, DG_total], F32)
                    nc.vector.memset(oz[:], 0.0)
                    nc.sync.dma_start(out[:], oz[:])

            # ---------------- decode ----------------
            if PH >= 5:
              with ExitStack() as ph:
                dp = ph.enter_context(tc.tile_pool(name="dp", bufs=4))
                dip = ph.enter_context(tc.tile_pool(name="dip", bufs=1))
                ds_sb = dip.tile([P, DG_total * 8], I16)
                nc.sync.dma_start(ds_sb[:], dsidx[:])
                dd_sb = dip.tile([P, DG_total * 8], I16)
                nc.sync.dma_start(dd_sb[:], ddidx[:])
                oacc = dip.tile([P, DG_total], F32)
                for (ca, cb, g0, gcount) in s["dec_runs"]:
                    for p0 in range(0, gcount, DEC_RING):
                        pc = min(DEC_RING, gcount - p0)
                        gg = g0 + p0
                        ms = dp.tile([P, DEC_RING, P], F32, tag="ds")
                        nc.gpsimd.dma_gather(
                            ms[:, :pc, :],
                            chunk_ap(tbl3, ca, s["chunk_rows"][ca]),
                            ds_sb[:, gg * 8:(gg + pc) * 8],
                            num_idxs=pc * P, num_idxs_reg=pc * P,
                            elem_size=HID, queue_num=next_q())
                        md = dp.tile([P, DEC_RING, P], F32, tag="dd")
                        nc.gpsimd.dma_gather(
                            md[:, :pc, :],
                            chunk_ap(tbl3, cb, s["chunk_rows"][cb]),
                            dd_sb[:, gg * 8:(gg + pc) * 8],
                            num_idxs=pc * P, num_idxs_reg=pc * P,
                            elem_size=HID, queue_num=next_q())
                        pr = dp.tile([P, DEC_RING, P], F32, tag="pr")
                        nc.vector.tensor_tensor(pr[:, :pc, :], ms[:, :pc, :],
                                                md[:, :pc, :],
                                                op=mybir.AluOpType.mult)
                        nc.vector.tensor_reduce(
                            oacc[:, gg:gg + pc], pr[:, :pc, :],
                            axis=mybir.AxisListType.X, op=mybir.AluOpType.add)
                nc.sync.dma_start(out[:], oacc[:])

    nc.compile()
    return nc


# ---------------------------------------------------------------- entry

def kernel(x, W1, b1, W2, b2, edge_index, edge_label_index):
    x = np.asarray(x)
    sched, in_maps, dec_orig = _prep(x, W1, b1, W2, b2,
                                     edge_index, edge_label_index)
    nc = _build(sched)
    res = run_bass_kernel_spmd(nc, in_maps, core_ids=list(range(NCORES)))
    NLAB = sched["NLAB"]
    outf = np.zeros(NLAB, dtype=np.float32)
    for c in range(NCORES):
        vals = res.results[c]["out"].T.ravel()       # stream order
        og = dec_orig[c]
        mreal = og >= 0
        outf[og[mreal]] = vals[mreal]
    return outf



# revision 9
# speedup vs baseline: 2.2576x; 2.2342x over previous
"""GCN link predictor on 8 trn2 NeuronCores (Bass/Tile).

Sharding: nodes row-sharded across 8 cores (dst-partitioned edges so the
segment-sum stays local); per-layer full feature tables shared via AllGather
(bf16, split into 4 row-pieces so transfers overlap the producing layer);
decode edges sharded data-parallel.

Math: gcn_conv(x) = D^-1/2 (A+I) D^-1/2 (x W) + b.  D^-1/2 is folded into the
tables: h' = (x W) * dinv[row], out[d] = dinv[d] * sum_{s in N(d)} h'[s] + b.
Aggregation = 0/1 select-matrix matmuls on TensorE over edge rows fetched by
dma_gather (bf16 256B rows, 4 SWDGE queues, <=1024 idxs/call — queue count and
call size are what set the ~1.7ns/row gather rate on this HW).  Self-loops
bypass the gather: psum += I @ h'_tile (dense local rows).  The +b is a rank-1
matmul (sqrt(deg) x b) in the same PSUM group; epilogue is one ScalarE
activation (scale=dinv or dinv^2, Relu/Identity).
"""
import math
import os
import numpy as np
from contextlib import ExitStack

import concourse.bass as bass
import concourse.bacc as bacc
import concourse.tile as tile
import concourse.mybir as mybir
from concourse.bass_utils import run_bass_kernel_spmd

F32 = mybir.dt.float32
BF16 = mybir.dt.bfloat16
I16 = mybir.dt.int16

NCORES = 8
P = 128
B_TILES = 6            # dst tiles per aggregation batch (PSUM banks: 6+1+1)
RING_G = int(os.environ.get("KRING", "8"))   # groups per dma_gather (<=8:
                                             # >1024 idxs per call faults)
DEC_RING = int(os.environ.get("KDRING", "8"))


def _rows_ap(handle, row0: int, nblk: int, width: int):
    """DRAM [rows, width] view as [128, nblk, width]: p,b,f -> row row0+b*128+p."""
    a = handle[:]
    return bass.AP(tensor=a.tensor, offset=row0 * width,
                   ap=[[width, 128], [128 * width, nblk], [1, width]])


# ---------------------------------------------------------------- host prep

def _bf16_round(a):
    """f32 -> bf16 (RNE), as an ml_dtypes.bfloat16 array."""
    import ml_dtypes
    u = np.ascontiguousarray(a, dtype=np.float32).view(np.uint32)
    r = ((u >> np.uint32(16)) & np.uint32(1)) + np.uint32(0x7FFF)
    return ((u + r) >> np.uint32(16)).astype(np.uint16).view(
        ml_dtypes.bfloat16)


def _pack_idx(stream_i16: np.ndarray) -> np.ndarray:
    """int16 stream -> [128, L/16] tile (pos i -> [i%16, i//16], x8 replicated)."""
    L = stream_i16.shape[0]
    assert L % 16 == 0
    a16 = stream_i16.reshape(L // 16, 16).T.copy()
    return np.tile(a16, (8, 1))


def _pack_stream(stream: np.ndarray) -> np.ndarray:
    """stream -> [128, L/128] tile (pos i -> [i%128, i//128])."""
    L = stream.shape[0]
    assert L % P == 0
    return stream.reshape(L // P, P).T.copy()


def _scatter_stream(order_vals, key_sorted, run_pad_starts, run_real_starts,
                    total_len, fill, dtype):
    """Place sorted run data into a padded stream."""
    out = np.full(total_len, fill, dtype=dtype)
    n = order_vals.shape[0]
    within = np.arange(n) - run_real_starts[key_sorted]
    out[run_pad_starts[key_sorted] + within] = order_vals
    return out


def _prep(x, W1, b1, W2, b2, edge_index, edge_label_index):
    N, CIN = x.shape
    HID = W1.shape[1]
    NLAB = edge_label_index.shape[1]
    NS = (N + NCORES - 1) // NCORES            # rows per shard (12500)
    NSP = ((NS + P - 1) // P) * P              # padded rows per shard
    PT = NSP // P                              # dst tiles per core

    # 4 row-pieces per shard (tile-aligned); chunk j = all cores' piece-j rows
    q, r = divmod(PT, 4)
    pieces_t = [q + (1 if i < r else 0) for i in range(4)]   # tiles per piece
    pr = [t * P for t in pieces_t]                           # rows/piece/core
    rs = np.concatenate([[0], np.cumsum(pr)])[:-1].astype(np.int64)
    NCH = 4
    chunk_rows = [NCORES * pr[j] for j in range(NCH)]
    assert max(NCORES * (prj - 1) + prj - 1 for prj in pr) < (1 << 15)
    piece_end_tile = np.cumsum(pieces_t)                     # [25, 50, 74, 98]

    n_batch = (PT + B_TILES - 1) // B_TILES

    src = np.asarray(edge_index[0], dtype=np.int64)
    dst = np.asarray(edge_index[1], dtype=np.int64)
    deg = np.bincount(dst, minlength=N).astype(np.float64) + 1.0
    dinv = (1.0 / np.sqrt(deg)).astype(np.float32)
    degh = np.sqrt(deg).astype(np.float32)

    rs_edges = np.append(rs, NSP)

    def tbl_row(v):
        cid = np.minimum(v // NS, NCORES - 1)
        rr = v - cid * NS
        p = np.searchsorted(rs_edges, rr, side="right") - 1
        rel = cid * np.asarray(pr)[p] + (rr - rs[p])
        return p, rel.astype(np.int16)

    # ---- aggregation streams (no self-loops; shared across both layers) ----
    NRUN = PT * NCH

    per_core = []
    run_lens = np.zeros((NCORES, NRUN), dtype=np.int64)
    for c in range(NCORES):
        lo, hi = c * NS, min((c + 1) * NS, N)
        m = (dst >= lo) & (dst < hi)
        es, edl = src[m], dst[m] - lo
        ch, rel = tbl_row(es)
        tl = edl // P
        drel = (edl - tl * P).astype(np.float32)
        key = (tl * NCH + ch).astype(np.int64)
        bkey = (tl // B_TILES) * (NCH * B_TILES) + ch * B_TILES + (tl % B_TILES)
        order = np.argsort(bkey, kind="stable")
        per_core.append((rel[order], drel[order], key[order]))
        run_lens[c] = np.bincount(key, minlength=NRUN)

    mx = run_lens.max(axis=0)
    G_tc = (((mx + P - 1) // P) * P // P).reshape(PT, NCH)  # groups/(tile,chunk)

    run_order = []   # (tile, chunk) in stream order
    for b in range(n_batch):
        tiles = list(range(b * B_TILES, min((b + 1) * B_TILES, PT)))
        for ch in range(NCH):
            for t in tiles:
                run_order.append((t, ch))
    pad_len = np.array([G_tc[t, ch] * P for (t, ch) in run_order], dtype=np.int64)
    pad_start_by_pos = np.concatenate([[0], np.cumsum(pad_len)])
    SL = int(pad_start_by_pos[-1])
    pos_of_key = np.zeros(NRUN, dtype=np.int64)
    for i, (t, ch) in enumerate(run_order):
        pos_of_key[t * NCH + ch] = i
    run_pad_starts = pad_start_by_pos[:-1][pos_of_key]      # by key

    eidx_tiles, edst_tiles = [], []
    for c in range(NCORES):
        rel_s, drel_s, key_s = per_core[c]
        rl = run_lens[c]
        order_keys = np.array([t * NCH + ch for (t, ch) in run_order])
        real_in_order = rl[order_keys]
        rstarts = np.concatenate([[0], np.cumsum(real_in_order)])[:-1]
        real_starts_by_key = np.zeros(NRUN, dtype=np.int64)
        real_starts_by_key[order_keys] = rstarts
        eidx = _scatter_stream(rel_s, key_s, run_pad_starts, real_starts_by_key,
                               SL, 0, np.int16)
        edst = _scatter_stream(drel_s, key_s, run_pad_starts,
                               real_starts_by_key, SL, -1.0, np.float32)
        eidx_tiles.append(_pack_idx(eidx))
        edst_tiles.append(_pack_stream(_bf16_round(edst)))

    G_total = SL // P

    # group -> tile map + per-(batch,chunk) extents, in stream order
    group_tile = np.empty(G_total, dtype=np.int64)
    batches = []   # per batch: (tiles, [(chunk, g0, gcount)])
    g = 0
    for b in range(n_batch):
        tiles = list(range(b * B_TILES, min((b + 1) * B_TILES, PT)))
        runs = []
        for ch in range(NCH):
            g0 = g
            for t in tiles:
                group_tile[g:g + G_tc[t, ch]] = t
                g += int(G_tc[t, ch])
            if g > g0:
                runs.append((ch, g0, g - g0))
        batches.append((tiles, runs))
    assert g == G_total

    # ---- decode streams ----
    DNS = (NLAB + NCORES - 1) // NCORES
    NCOMBO = NCH * NCH
    combo_rank = sorted(range(NCOMBO),
                        key=lambda cm: (max(cm // NCH, cm % NCH),
                                        cm // NCH, cm % NCH))
    rank_of = np.zeros(NCOMBO, dtype=np.int64)
    for i, cm in enumerate(combo_rank):
        rank_of[cm] = i
    dec_lens = np.zeros((NCORES, NCOMBO), dtype=np.int64)
    dec_core = []
    ls = np.asarray(edge_label_index[0], dtype=np.int64)
    ld = np.asarray(edge_label_index[1], dtype=np.int64)
    for c in range(NCORES):
        lo, hi = c * DNS, min((c + 1) * DNS, NLAB)
        ca, sa = tbl_row(ls[lo:hi])
        cb, sb = tbl_row(ld[lo:hi])
        combo = (ca * NCH + cb).astype(np.int64)
        order = np.argsort(rank_of[combo], kind="stable")
        dec_core.append((sa[order], sb[order], combo[order],
                         np.arange(lo, hi, dtype=np.int64)[order]))
        dec_lens[c] = np.bincount(combo, minlength=NCOMBO)

    mxd = dec_lens.max(axis=0)
    dec_pad = ((mxd + P - 1) // P) * P                      # by combo id
    # padded starts follow combo_rank order
    dec_starts_by_rank = np.concatenate(
        [[0], np.cumsum(dec_pad[np.asarray(combo_rank)])])
    DSL = int(dec_starts_by_rank[-1])
    DG_total = DSL // P
    dec_pad_start = np.zeros(NCOMBO, dtype=np.int64)
    for i, cm in enumerate(combo_rank):
        dec_pad_start[cm] = dec_starts_by_rank[i]

    ds_tiles, dd_tiles, dec_orig = [], [], []
    for c in range(NCORES):
        s_rel, d_rel, combo_s, orig_s = dec_core[c]
        rl = dec_lens[c]
        rstarts = np.zeros(NCOMBO, dtype=np.int64)
        rstarts[np.asarray(combo_rank)] = np.concatenate(
            [[0], np.cumsum(rl[np.asarray(combo_rank)])])[:-1]
        ds = _scatter_stream(s_rel, combo_s, dec_pad_start, rstarts,
                             DSL, 0, np.int16)
        dd = _scatter_stream(d_rel, combo_s, dec_pad_start, rstarts,
                             DSL, 0, np.int16)
        og = _scatter_stream(orig_s, combo_s, dec_pad_start, rstarts,
                             DSL, -1, np.int64)
        ds_tiles.append(_pack_idx(ds))
        dd_tiles.append(_pack_idx(dd))
        dec_orig.append(og)
    dec_runs = []   # (chunk_a, chunk_b, g0, gcount) in rank order
    for cm in combo_rank:
        if dec_pad[cm]:
            dec_runs.append((cm // NCH, cm % NCH,
                             int(dec_pad_start[cm]) // P, int(dec_pad[cm]) // P))

    # ---- dense inputs ----
    xT = np.asarray(x, dtype=np.float32).T                 # [CIN, N]
    xT_shards, dinv_tiles, dinv2_tiles, degh_rows = [], [], [], []
    for c in range(NCORES):
        lo, hi = c * NS, min((c + 1) * NS, N)
        sh = np.zeros((CIN, NSP), dtype=np.float32)
        sh[:, : hi - lo] = xT[:, lo:hi]
        xT_shards.append(_bf16_round(sh))
        dv = np.ones(NSP, dtype=np.float32)
        dv[: hi - lo] = dinv[lo:hi]
        dinv_tiles.append(_pack_stream(dv))                 # [128, PT] f32
        dinv2_tiles.append(_pack_stream(dv * dv))
        dg = np.zeros((1, NSP), dtype=np.float32)
        dg[0, : hi - lo] = degh[lo:hi]
        degh_rows.append(_bf16_round(dg))

    iota = np.tile(np.arange(P, dtype=np.float32), (P, 1))
    ident = np.eye(P, dtype=np.float32)
    b1r = np.asarray(b1, dtype=np.float32).reshape(1, HID)
    b2r = np.asarray(b2, dtype=np.float32).reshape(1, HID)

    sched = dict(
        N=N, CIN=CIN, HID=HID, NS=NS, NSP=NSP, PT=PT, NCH=NCH,
        chunk_rows=chunk_rows, piece_rows=pr, piece_starts=list(rs),
        piece_end_tile=list(piece_end_tile),
        n_batch=n_batch, batches=batches,
        group_tile=group_tile, G_total=G_total, dec_runs=dec_runs,
        DG_total=DG_total, NLAB=NLAB, DNS=DNS,
    )
    inputs = [dict(
        xT=xT_shards[c], eidx=eidx_tiles[c], edst=edst_tiles[c],
        dsidx=ds_tiles[c], ddidx=dd_tiles[c], dinv_t=dinv_tiles[c],
        dinv2_t=dinv2_tiles[c], degh=degh_rows[c],
        W1=_bf16_round(W1), W2=_bf16_round(W2),
        b1r=_bf16_round(b1r), b2r=_bf16_round(b2r),
        iota=_bf16_round(iota), ident=_bf16_round(ident),
    ) for c in range(NCORES)]
    return sched, inputs, dec_orig


# ---------------------------------------------------------------- device

def _build(s):
    CIN, HID, NSP, PT, NCH = s["CIN"], s["HID"], s["NSP"], s["PT"], s["NCH"]
    G_total, DG_total = s["G_total"], s["DG_total"]
    NQ = int(os.environ.get("KNQ", "4"))
    KSIM = os.environ.get("KSIM", "") == "1"   # single-core collective-free
    nc = bacc.Bacc("TRN2", target_bir_lowering=False, debug=False,
                   num_devices=1 if KSIM else NCORES, num_swdge_queues=NQ)
    qctr = [0]

    def next_q():
        qctr[0] += 1
        return qctr[0] % NQ

    xT = nc.dram_tensor("xT", [CIN, NSP], BF16, kind="ExternalInput")
    eidx = nc.dram_tensor("eidx", [P, G_total * 8], I16, kind="ExternalInput")
    edst = nc.dram_tensor("edst", [P, G_total], BF16, kind="ExternalInput")
    dsidx = nc.dram_tensor("dsidx", [P, DG_total * 8], I16, kind="ExternalInput")
    ddidx = nc.dram_tensor("ddidx", [P, DG_total * 8], I16, kind="ExternalInput")
    dinv_t = nc.dram_tensor("dinv_t", [P, PT], F32, kind="ExternalInput")
    dinv2_t = nc.dram_tensor("dinv2_t", [P, PT], F32, kind="ExternalInput")
    degh = nc.dram_tensor("degh", [1, NSP], BF16, kind="ExternalInput")
    W1d = nc.dram_tensor("W1", [CIN, HID], BF16, kind="ExternalInput")
    W2d = nc.dram_tensor("W2", [HID, HID], BF16, kind="ExternalInput")
    b1d = nc.dram_tensor("b1r", [1, HID], BF16, kind="ExternalInput")
    b2d = nc.dram_tensor("b2r", [1, HID], BF16, kind="ExternalInput")
    iotad = nc.dram_tensor("iota", [P, P], BF16, kind="ExternalInput")
    identd = nc.dram_tensor("ident", [P, P], BF16, kind="ExternalInput")

    out = nc.dram_tensor("out", [P, DG_total], F32, kind="ExternalOutput")

    hp_sh = nc.dram_tensor("hp_sh", [NSP, HID], BF16)
    h2_sh = nc.dram_tensor("h2_sh", [NSP, HID], BF16)
    z_sh = nc.dram_tensor("z_sh", [NSP, HID], BF16)
    PRS, PST = s["piece_rows"], s["piece_starts"]

    def mk_table(name):
        return [nc.dram_tensor(f"{name}_p{j}", [s["chunk_rows"][j], HID],
                               BF16, addr_space="Shared") for j in range(NCH)]

    tbl1, tbl2, tbl3 = mk_table("tbl1"), mk_table("tbl2"), mk_table("tbl3")
    groups = [list(range(NCORES))]

    def ag_piece(sh, tabs, j):
        i0, nrow = PST[j], PRS[j]
        if KSIM:
            for c in range(NCORES):
                nc.sync.dma_start(tabs[j][c * nrow:(c + 1) * nrow, :],
                                  sh[i0:i0 + nrow, :])
            return
        nc.gpsimd.collective_compute(
            "AllGather", mybir.AluOpType.bypass, replica_groups=groups,
            ins=[sh[i0:i0 + nrow, :]], outs=[tabs[j][:]])

    KT = CIN // P   # k chunks for layer-1 matmul
    PH = int(os.environ.get("KPHASE", "4"))
    AGGMODE = os.environ.get("AGGMODE", "full")

    with tile.TileContext(nc) as tc:
        with ExitStack() as root:
            cp = root.enter_context(tc.tile_pool(name="const", bufs=1))
            W1_sb = cp.tile([P, KT * HID], BF16)
            for k in range(KT):
                nc.sync.dma_start(W1_sb[:, k * HID:(k + 1) * HID],
                                  W1d[k * P:(k + 1) * P, :])
            W2_sb = cp.tile([P, HID], BF16)
            nc.sync.dma_start(W2_sb[:], W2d[:])
            b1_sb = cp.tile([1, HID], BF16)
            nc.sync.dma_start(b1_sb[:], b1d[:])
            b2_sb = cp.tile([1, HID], BF16)
            nc.sync.dma_start(b2_sb[:], b2d[:])
            iota_sb = cp.tile([P, P], BF16)
            nc.sync.dma_start(iota_sb[:], iotad[:])
            ident_sb = cp.tile([P, P], BF16)
            nc.sync.dma_start(ident_sb[:], identd[:])
            dinv_sb = cp.tile([P, PT], F32)
            nc.sync.dma_start(dinv_sb[:], dinv_t[:])
            dinv2_sb = cp.tile([P, PT], F32)
            nc.sync.dma_start(dinv2_sb[:], dinv2_t[:])
            degh_sb = cp.tile([1, NSP], BF16)
            nc.sync.dma_start(degh_sb[:], degh[:])
            eidx_sb = cp.tile([P, G_total * 8], I16)
            nc.sync.dma_start(eidx_sb[:], eidx[:])
            edst_sb = cp.tile([P, G_total], BF16)
            nc.sync.dma_start(edst_sb[:], edst[:])

            # ---------------- phase 1: hp = (x @ W1) * dinv ----------------
            with ExitStack() as ph:
                xp = ph.enter_context(tc.tile_pool(name="xp", bufs=3))
                op = ph.enter_context(tc.tile_pool(name="op", bufs=2))
                pp = ph.enter_context(tc.tile_pool(name="pp", bufs=4,
                                                   space="PSUM"))
                OB = 8   # tiles per input/output DMA batch
                issued = [False] * NCH
                for blk in range(0, PT, OB):
                    nt = min(OB, PT - blk)
                    ob = op.tile([P, OB * HID], BF16, tag="hpout")
                    xt = xp.tile([P, KT, OB * P], BF16, tag="xt")
                    for k in range(KT):
                        nc.sync.dma_start(
                            xt[:, k, :nt * P],
                            xT[k * P:(k + 1) * P, blk * P:(blk + nt) * P])
                    for j in range(nt):
                        t = blk + j
                        ps = pp.tile([P, HID], F32, tag="p1")
                        for k in range(KT):
                            nc.tensor.matmul(
                                out=ps[:], lhsT=xt[:, k, j * P:(j + 1) * P],
                                rhs=W1_sb[:, k * HID:(k + 1) * HID],
                                start=(k == 0), stop=(k == KT - 1))
                        nc.scalar.activation(
                            ob[:, j * HID:(j + 1) * HID], ps[:],
                            mybir.ActivationFunctionType.Identity,
                            scale=dinv_sb[:, t:t + 1])
                    dr = _rows_ap(hp_sh, blk * P, nt, HID)
                    nc.sync.dma_start(dr, ob[:, :nt * HID].rearrange(
                        "p (b f) -> p b f", b=nt))
                    if PH >= 2:
                        for j in range(NCH):
                            if not issued[j] and blk + nt >= s["piece_end_tile"][j]:
                                ag_piece(hp_sh, tbl1, j)
                                issued[j] = True

            # ---------------- aggregation layers ----------------
            def agg_layer(table, brow, dscale, relu, l2_tail, self_sh, out_sh,
                          next_sh, next_tabs):
                with ExitStack() as ph:
                    mp = ph.enter_context(tc.tile_pool(name="mp", bufs=10))
                    sp = ph.enter_context(tc.tile_pool(name="sp", bufs=4))
                    zp = ph.enter_context(tc.tile_pool(name="zp", bufs=2))
                    hp = ph.enter_context(tc.tile_pool(name="hp", bufs=2))
                    ap = ph.enter_context(tc.tile_pool(name="ap", bufs=1,
                                                       space="PSUM"))
                    issued = [False] * NCH
                    for bi, (tiles, runs) in enumerate(s["batches"]):
                        nt = len(tiles)
                        # dense local rows for the self-loop term
                        hs = hp.tile([P, B_TILES * HID], BF16, tag="self")
                        nc.sync.dma_start(
                            hs[:, :nt * HID].rearrange("p (b f) -> p b f",
                                                       b=nt),
                            _rows_ap(self_sh, tiles[0] * P, nt, HID))
                        psums = {}
                        started = set()
                        for (ch, g0, gcount) in runs:
                            rows = s["chunk_rows"][ch]
                            for p0 in range(0, gcount, RING_G):
                                pc = min(RING_G, gcount - p0)
                                gg = g0 + p0
                                m = mp.tile([P, RING_G, P], BF16, tag="msg")
                                if "nogather" not in AGGMODE:
                                    nc.gpsimd.dma_gather(
                                        m[:, :pc, :], table[ch][0:rows, :],
                                        eidx_sb[:, gg * 8:(gg + pc) * 8],
                                        num_idxs=pc * P, num_idxs_reg=pc * P,
                                        elem_size=HID, queue_num=next_q())
                                # sel[p,q,j] = (iota[p,j] == edst[p,gg+q])
                                selb = sp.tile([P, RING_G, P], BF16, tag="sel")
                                ia = iota_sb[:]
                                iota_b = bass.AP(
                                    tensor=ia.tensor, offset=ia.offset,
                                    ap=[list(ia.ap[0]), [0, pc],
                                        list(ia.ap[1])])
                                ea = edst_sb[:, gg:gg + pc]
                                edst_b = bass.AP(
                                    tensor=ea.tensor, offset=ea.offset,
                                    ap=[list(ea.ap[0]), list(ea.ap[1]),
                                        [0, P]])
                                nc.vector.tensor_tensor(
                                    selb[:, :pc, :], iota_b, edst_b,
                                    op=mybir.AluOpType.is_equal)
                                for q in range(pc):
                                    gq = gg + q
                                    t = int(s["group_tile"][gq])
                                    if t not in psums:
                                        psums[t] = ap.tile(
                                            [P, HID], F32, name=f"psum{t}",
                                            tag=f"acc{t % B_TILES}")
                                    nc.tensor.matmul(
                                        out=psums[t][:], lhsT=selb[:, q, :],
                                        rhs=m[:, q, :],
                                        start=(t not in started), stop=False,
                                        skip_group_check=True)
                                    started.add(t)
                        ob = zp.tile([P, B_TILES * HID], BF16, tag="zout")
                        for j, t in enumerate(tiles):
                            if t not in psums:
                                psums[t] = ap.tile([P, HID], F32,
                                                   name=f"psum{t}",
                                                   tag=f"acc{t % B_TILES}")
                                started.discard(t)
                            # self-loop: psum += I @ h'_tile
                            nc.tensor.matmul(
                                out=psums[t][:], lhsT=ident_sb[:],
                                rhs=hs[:, j * HID:(j + 1) * HID],
                                start=(t not in started), stop=False,
                                skip_group_check=True)
                            # bias: psum += sqrt(deg) x b
                            nc.tensor.matmul(
                                out=psums[t][:],
                                lhsT=degh_sb[:1, t * P:(t + 1) * P],
                                rhs=brow[:1, :],
                                start=False, stop=True, skip_group_check=True)
                            if l2_tail:
                                zt = sp.tile([P, HID], BF16, tag="zt")
                                nc.scalar.activation(
                                    zt[:], psums[t][:],
                                    mybir.ActivationFunctionType.Relu,
                                    scale=dscale[:, t:t + 1])
                                trp = ap.tile([P, P], BF16, tag="tr")
                                nc.tensor.transpose(trp[:], zt[:],
                                                    ident_sb[:])
                                a1t = sp.tile([P, P], BF16, tag="a1t")
                                nc.vector.tensor_copy(a1t[:], trp[:])
                                h2p = ap.tile([P, HID], F32, tag="h2")
                                nc.tensor.matmul(out=h2p[:], lhsT=a1t[:],
                                                 rhs=W2_sb[:], start=True,
                                                 stop=True)
                                nc.vector.tensor_copy(
                                    ob[:, j * HID:(j + 1) * HID], h2p[:])
                            else:
                                nc.scalar.activation(
                                    ob[:, j * HID:(j + 1) * HID], psums[t][:],
                                    mybir.ActivationFunctionType.Identity,
                                    scale=dscale[:, t:t + 1])
                        dr = _rows_ap(out_sh, tiles[0] * P, nt, HID)
                        nc.sync.dma_start(dr, ob[:, :nt * HID].rearrange(
                            "p (b f) -> p b f", b=nt))
                        if next_tabs is not None:
                            done_t = tiles[-1] + 1
                            for j in range(NCH):
                                if not issued[j] and \
                                        done_t >= s["piece_end_tile"][j]:
                                    ag_piece(next_sh, next_tabs, j)
                                    issued[j] = True

            if PH >= 2:
                agg_layer(tbl1, b1_sb, dinv2_sb, relu=True, l2_tail=True,
                          self_sh=hp_sh, out_sh=h2_sh,
                          next_sh=h2_sh, next_tabs=tbl2 if PH >= 3 else None)
            if PH >= 3:
                agg_layer(tbl2, b2_sb, dinv_sb, relu=False, l2_tail=False,
                          self_sh=h2_sh, out_sh=z_sh,
                          next_sh=z_sh, next_tabs=tbl3 if PH >= 4 else None)

            if PH < 4:
                with ExitStack() as ph:
                    zp0 = ph.enter_context(tc.tile_pool(name="zp0", bufs=1))
                    oz = zp0.tile([P, DG_total], F32)
                    nc.vector.memset(oz[:], 0.0)
                    nc.sync.dma_start(out[:], oz[:])

            # ---------------- decode ----------------
            if PH >= 4:
              with ExitStack() as ph:
                dp = ph.enter_context(tc.tile_pool(name="dp", bufs=6))
                dip = ph.enter_context(tc.tile_pool(name="dip", bufs=1))
                ds_sb = dip.tile([P, DG_total * 8], I16)
                nc.sync.dma_start(ds_sb[:], dsidx[:])
                dd_sb = dip.tile([P, DG_total * 8], I16)
                nc.sync.dma_start(dd_sb[:], ddidx[:])
                oacc = dip.tile([P, DG_total], F32)
                for (ca, cb, g0, gcount) in s["dec_runs"]:
                    for p0 in range(0, gcount, DEC_RING):
                        pc = min(DEC_RING, gcount - p0)
                        gg = g0 + p0
                        ms = dp.tile([P, DEC_RING, P], BF16, tag="ds")
                        nc.gpsimd.dma_gather(
                            ms[:, :pc, :],
                            tbl3[ca][0:s["chunk_rows"][ca], :],
                            ds_sb[:, gg * 8:(gg + pc) * 8],
                            num_idxs=pc * P, num_idxs_reg=pc * P,
                            elem_size=HID, queue_num=next_q())
                        md = dp.tile([P, DEC_RING, P], BF16, tag="dd")
                        nc.gpsimd.dma_gather(
                            md[:, :pc, :],
                            tbl3[cb][0:s["chunk_rows"][cb], :],
                            dd_sb[:, gg * 8:(gg + pc) * 8],
                            num_idxs=pc * P, num_idxs_reg=pc * P,
                            elem_size=HID, queue_num=next_q())
                        pr_t = dp.tile([P, DEC_RING, P], BF16, tag="pr")
                        nc.vector.tensor_tensor(pr_t[:, :pc, :],
                                                ms[:, :pc, :], md[:, :pc, :],
                                                op=mybir.AluOpType.mult)
                        nc.vector.tensor_reduce(
                            oacc[:, gg:gg + pc], pr_t[:, :pc, :],
                            axis=mybir.AxisListType.X, op=mybir.AluOpType.add)
                nc.sync.dma_start(out[:], oacc[:])

    nc.compile()
    return nc


# ---------------------------------------------------------------- entry

def kernel(x, W1, b1, W2, b2, edge_index, edge_label_index):
    x = np.asarray(x)
    sched, in_maps, dec_orig = _prep(x, W1, b1, W2, b2,
                                     edge_index, edge_label_index)
    nc = _build(sched)
    res = run_bass_kernel_spmd(nc, in_maps, core_ids=list(range(NCORES)))
    NLAB = sched["NLAB"]
    outf = np.zeros(NLAB, dtype=np.float32)
    for c in range(NCORES):
        vals = res.results[c]["out"].T.ravel()       # stream order
        og = dec_orig[c]
        mreal = og >= 0
        outf[og[mreal]] = vals[mreal]
    return outf
